# revision 6
# baseline (speedup 1.0000x reference)
"""Taylor feature map kernel for Trainium2 (Bass/Tile), 8-core SPMD.

Input  x:   (2, 16, 2048, 64) f32  -> 65536 rows of dim 64
Output out: (2, 16, 2048, 2145) f32 per row:
    [1, x/D^0.25, x_i^2/(sqrt(D)*sqrt(2)), x_i*x_j/sqrt(D) for i<j (row-major)]

Strategy (v3, ~114us -> target <100us):
- Device emits the 2016 pair products as bf16 (rel-err gate 2e-2 admits it);
  ones/linear/diag (129 of 2145 cols) stay on the host in exact f32; input
  is prescaled bf16.  The kernel is co-limited by the HBM store stream
  (~432GB/s = the 16 DMA engines' aggregate) and the DVE (2x_1p
  tensor_tensor: 2 elem/cycle/lane, ~0.52ns/elem + ~65ns/op).
- CYCLIC-shift decomposition (v3): for s=1..31, out_s[i] = x[i] *
  x[(i+s) mod 64] gives 64 valid products per shift (the wrapped tail
  covers difference 64-s), plus a 32-wide half block for s=32:
  31*64 + 32 = 2016 columns, ZERO pads (v2 had 92), and only 9 DVE ops
  per supertile (v2 had 17):
  * y tile [x|x-wrap (96) | (x>>1)-wrap (96)]: even shifts s read y_even
    at offset s (even -> 4B-aligned); odd shifts s=2t+1 read y_odd at
    offset 2t (even).  One 4D-AP op covers 4 same-parity shifts (out
    row-stride 128 cols, operand row-stride 2), keeping 2x_1p packing
    (2B dtype, stride-1 inner, 4B-aligned sub-rows).
  * DVE payload drops 4.4% (no pad lanes), store bytes drop 4.4%
    (33.03MB vs 34.54MB per core), op overhead nearly halves.
- Pipeline: one cast chain per supertile on the Scalar engine (4 AP
  pieces); input DMAs on the idle GpSimd queue; stores on the Sync queue;
  supertile schedule [2,4,4,6,8x6] primes the store stream by ~15us
  (DVE production rate ~458GB/s now exceeds the stream, so the queue
  stays fed); apool bufs=4 decouples DVE from store-queue lag.
"""

import math
from contextlib import ExitStack

import numpy as np

try:
    import concourse.bass as bass
except ImportError:  # container path for the concourse framework
    import sys

    sys.path.insert(0, "/opt/trn_rl_repo")
    import concourse.bass as bass

import concourse.mybir as mybir
from concourse import tile
from concourse.bass_utils import run_bass_kernel_spmd
from concourse.vector_clock import ScopedClock

MAX_WAITS = 1


class SplitWaitTileContext(tile.TileContext):
    """The stock walrus in this environment rejects instructions carrying
    more than one sync wait ("Too many sync wait commands", observed for
    both TPB_CTRL Drain and DMA_DIRECT2D). Hoist excess waits onto NoOp
    carrier instructions committed just before, on the same engine queue."""

    def _split_waits(self, inst):
        si = getattr(inst, "sync_info", None)
        eng = getattr(inst, "engine", None)
        if (
            si is None
            or not si.on_wait
            or len(si.on_wait) <= MAX_WAITS
            or eng is None
            or eng == mybir.EngineType.Unassigned
        ):
            return None
        waits = list(si.on_wait)
        extra, keep = waits[:-MAX_WAITS], waits[-MAX_WAITS:]
        inst.sync_info = mybir.SyncInfo(on_wait=keep,
                                        on_update=list(si.on_update))
        nops = []
        for i in range(0, len(extra), MAX_WAITS):
            nops.append(mybir.InstNoOp(
                name=self.nc.get_next_instruction_name(),
                sync_info=mybir.SyncInfo(on_wait=extra[i:i + MAX_WAITS],
                                         on_update=[]),
                bass_nofuse=True,
                engine=eng,
            ))
        return nops

    def _commit_instruction(self, inst, lazy_reg_writes=True):
        if isinstance(inst, mybir.Instruction):
            nops = self._split_waits(inst)
            if nops:
                for nop in nops:
                    super()._commit_instruction(nop)
        return super()._commit_instruction(inst, lazy_reg_writes)

    def _drain_and_barrier(self, tick_clock, wait_clock):
        nc = self.nc
        drain_inst = nc.sync.drain()
        wait_clock.add_sem_waits(
            drain_inst.ins, ScopedClock({None: tick_clock.global_clock})
        )
        nops = self._split_waits(drain_inst.ins)
        if nops:
            # _commit path is closed here; append carriers directly, then
            # re-emit a drain that executes after them on the same queue.
            for nop in nops:
                self._add_instruction(nop)
            nc.sync.drain()

        nc.all_engine_barrier()
        assert self.sems is not None
        popped = nc._tile_sem_poison_stack.pop()
        assert popped is self._sem_poison
        nc.clear_and_free_semaphores(list(self.sems.allocated().values()))
        nc.all_engine_barrier()

D = 64
N_CROSS = (D * (D - 1)) // 2  # 2016
OUT_D = 1 + D + D + N_CROSS   # 2145
P = 128
N_CORES = 8
ROWS_TOTAL = 2 * 16 * 2048    # 65536
ROWS_PER_CORE = ROWS_TOTAL // N_CORES  # 8192

RD = math.sqrt(D)                      # 8.0
RRD_INV = 1.0 / math.sqrt(RD)          # 1/D^0.25
DIAG_C = 1.0 / (RD * math.sqrt(2.0))
PRESCALE = 1.0 / math.sqrt(RD)         # y = x*PRESCALE -> y_i*y_j = x_i*x_j/rd

G_ALL = 64                    # row-groups per partition (8192 rows / 128)
G_SCHED = [2, 4, 4, 6, 8, 8, 8, 8, 8, 8]  # supertile heights, sum = 64
assert sum(G_SCHED) == G_ALL

# device column layout: shift-s block at (s-1)*64, width 64 (s=1..31);
# s=32 half block at 1984, width 32.  col (s-1)*64 + i holds
# x_i * x_{(i+s) mod 64}.  Zero pads.
DEV_COLS = 31 * D + 32        # 2016
YW = 192                      # y tile: [x|x-wrap (96) | (x>>1)-wrap (96)]
YO = 96                       # offset of the odd copy

# DVE op groups: 4 same-parity shifts per op (3 for the last even group).
# (i1_off, out_base, c): i1 row-stride 2, out row-stride 128.
OP_GROUPS = []
for _t0 in (0, 4, 8, 12):          # odd shifts s = 2t+1, t = t0..t0+3
    OP_GROUPS.append((YO + 2 * _t0, 128 * _t0, 4))
for _t0, _c in ((1, 4), (5, 4), (9, 4), (13, 3)):  # even shifts s = 2t
    OP_GROUPS.append((2 * _t0, (2 * _t0 - 1) * D, _c))

# host gather map: reference cross column q (triu order) -> device column
_iu, _ju = np.triu_indices(D, k=1)
SRC_COLS = np.empty(N_CROSS, np.int64)
for _q in range(N_CROSS):
    _i, _j = int(_iu[_q]), int(_ju[_q])
    _d = _j - _i
    if _d < 32:
        SRC_COLS[_q] = (_d - 1) * D + _i
    elif _d == 32:
        SRC_COLS[_q] = 1984 + _i
    else:
        SRC_COLS[_q] = (64 - _d - 1) * D + _j


def build_nc() -> bass.Bass:
    nc = bass.Bass()
    x = nc.declare_dram_parameter("x", [P, G_ALL * D], mybir.dt.bfloat16,
                                  isOutput=False)
    out = nc.declare_dram_parameter("out", [ROWS_PER_CORE, DEV_COLS],
                                    mybir.dt.bfloat16, isOutput=True)

    bf16 = mybir.dt.bfloat16
    AF = mybir.ActivationFunctionType

    with SplitWaitTileContext(nc) as tc, ExitStack() as ctx:
        xp = ctx.enter_context(tc.tile_pool(name="xp", bufs=1))
        yp = ctx.enter_context(tc.tile_pool(name="yp", bufs=4))
        apool = ctx.enter_context(tc.tile_pool(name="apool", bufs=4))

        # per-ST input slabs [P, G*64]: one contiguous DMA per partition.
        # GpSimd queue keeps the Scalar queue free for the casts.
        x_tiles = []
        g0 = 0
        for i, g in enumerate(G_SCHED):
            xt = xp.tile([P, g * D], bf16, tag="x", bufs=4, name=f"x_sb{i}")
            nc.gpsimd.dma_start(xt[:], x[:, g0 * D:(g0 + g) * D])
            x_tiles.append(xt)
            g0 += g
        out_v = out.rearrange("(p g) d -> p g d", g=G_ALL)

        g0 = 0
        for st, groups in enumerate(G_SCHED):
            xs = x_tiles[st]
            xs_t = xs[:, 0:1]
            xs_ps = xs_t.ap[0][0]
            # y = [x|x-wrap | (x>>1)-wrap] built by 4 AP pieces on ACT:
            #   y[g, 0:64]    = x[g, 0:64]
            #   y[g, 64:96]   = x[g, 0:32]     (wrap)
            #   y[g, 96:159]  = x[g, 1:64]     (odd copy)
            #   y[g, 159:191] = x[g, 0:32]     (odd wrap)
            y = yp.tile([P, groups, YW], bf16, tag="y")
            y_t = y[:, :, 0:1]
            y_ps = y_t.ap[0][0]
            for y_off, x_off, w in ((0, 0, D), (D, 0, 32),
                                    (YO, 1, D - 1), (YO + D - 1, 0, 32)):
                o = bass.AP(y_t.tensor, y_off,
                            [[y_ps, P], [YW, groups], [1, w]])
                i_ = bass.AP(xs_t.tensor, x_off,
                             [[xs_ps, P], [D, groups], [1, w]])
                nc.scalar.activation(o, i_, AF.Copy)

            a_sb = apool.tile([P, groups, DEV_COLS], bf16, tag="a")
            a_t = a_sb[:, :, 0:1]
            a_ps = a_t.ap[0][0]

            # 8 group ops x 4 (or 3) same-parity shifts
            for i1_off, out_base, c in OP_GROUPS:
                o = bass.AP(a_t.tensor, out_base,
                            [[a_ps, P], [DEV_COLS, groups], [2 * D, c],
                             [1, D]])
                i0 = bass.AP(y_t.tensor, 0,
                             [[y_ps, P], [YW, groups], [0, c], [1, D]])
                i1 = bass.AP(y_t.tensor, i1_off,
                             [[y_ps, P], [YW, groups], [2, c], [1, D]])
                nc.vector.tensor_mul(o, i0, i1)
            # s=32 half block
            o = bass.AP(a_t.tensor, 1984,
                        [[a_ps, P], [DEV_COLS, groups], [1, 32]])
            i0 = bass.AP(y_t.tensor, 0, [[y_ps, P], [YW, groups], [1, 32]])
            i1 = bass.AP(y_t.tensor, 32, [[y_ps, P], [YW, groups], [1, 32]])
            nc.vector.tensor_mul(o, i0, i1)

            # split 120+8 partitions: HWDGE round-robins descriptors over
            # engines 64..79 restarting per DMA, so engine 79 (which also
            # serves as the dynamic queues' descriptor engine and runs ~20%
            # slower here) gets 7 instead of 8 descriptors per supertile.
            nc.sync.dma_start(out_v[0:120, g0:g0 + groups, :], a_sb[0:120])
            nc.sync.dma_start(out_v[120:P, g0:g0 + groups, :], a_sb[120:P])
            g0 += groups
    return nc


_NC_CACHE: dict = {}


def _install_ntff_hook_shim():
    """The image's antenv lacks axon_hooks; provide it so trace=True can
    drive NRT profiling via ctypes into libaxon_pjrt.so."""
    import sys as _sys
    import types
    import ctypes
    import contextlib

    if "antenv.axon_hooks" in _sys.modules:
        return
    so_path = "/opt/axon/libaxon_pjrt.so"
    lib = ctypes.CDLL(so_path)
    if not hasattr(lib, "axon_start_nrt_profile"):
        return
    lib.axon_start_nrt_profile.argtypes = [
        ctypes.POINTER(ctypes.c_int64), ctypes.c_size_t]
    lib.axon_start_nrt_profile.restype = ctypes.c_int64
    lib.axon_stop_nrt_profile.argtypes = [ctypes.c_char_p]
    lib.axon_stop_nrt_profile.restype = ctypes.c_int64

    @contextlib.contextmanager
    def _hook(output_dir, device_ids):
        import jax
        jax.devices()
        if device_ids:
            ids = (ctypes.c_int64 * len(device_ids))(*device_ids)
            rc = lib.axon_start_nrt_profile(ids, len(device_ids))
        else:
            rc = lib.axon_start_nrt_profile(None, 0)
        if rc != 0:
            raise RuntimeError(f"axon_start_nrt_profile rc={rc}")
        try:
            yield
        finally:
            n = lib.axon_stop_nrt_profile(str(output_dir).encode())
            print(f"ntff profile: {n} file(s) written to {output_dir}")

    mod = types.ModuleType("antenv.axon_hooks")
    mod.set_axon_ntff_profile_hook = lambda h: None
    mod.get_axon_ntff_profile_hook = lambda: _hook
    _sys.modules["antenv.axon_hooks"] = mod
    import antenv
    antenv.axon_hooks = mod


def _get_nc():
    if "nc" not in _NC_CACHE:
        _NC_CACHE["nc"] = build_nc()
    return _NC_CACHE["nc"]


def _install_loud_cc_hook():
    """Surface the real python traceback when the PJRT compile callback
    fails (the C++ caller swallows it)."""
    from concourse import bass2jax
    bass2jax.install_neuronx_cc_hook()
    try:
        import libneuronxla
    except ImportError:
        return
    if getattr(libneuronxla, "_loud_wrapped", False):
        return
    orig = libneuronxla.neuronx_cc

    def loud_hook(*a, **kw):
        try:
            return orig(*a, **kw)
        except BaseException:
            import traceback
            import sys as _s
            traceback.print_exc()
            _s.stderr.flush()
            raise

    libneuronxla.neuronx_cc = loud_hook
    libneuronxla._loud_wrapped = True
    bass2jax.install_neuronx_cc_hook = lambda: None


def _assemble(x_rows: np.ndarray, dev_rows: np.ndarray) -> np.ndarray:
    """Host assembly: exact f32 ones/linear/diag + permuted bf16 cross."""
    rows = x_rows.shape[0]
    full = np.empty((rows, OUT_D), np.float32)
    full[:, 0] = 1.0
    np.multiply(x_rows, np.float32(RRD_INV), out=full[:, 1:1 + D])
    np.multiply(np.square(x_rows), np.float32(DIAG_C),
                out=full[:, 1 + D:1 + 2 * D])
    # gather in bf16 (cheap), cast on assignment
    full[:, 1 + 2 * D:] = dev_rows[:, SRC_COLS]
    return full


def _run(x_rows: np.ndarray, trace: bool = False):
    """x_rows: [65536, 64] f32 (unscaled). Returns (full_out_rows, res)."""
    _install_loud_cc_hook()
    if trace:
        _install_ntff_hook_shim()
    nc = _get_nc()
    import ml_dtypes
    xc = np.ascontiguousarray(
        (x_rows * np.float32(PRESCALE)).astype(ml_dtypes.bfloat16))
    xs = xc.reshape(N_CORES, P, G_ALL * D)
    in_maps = [{"x": np.ascontiguousarray(xs[c])} for c in range(N_CORES)]
    res = run_bass_kernel_spmd(nc, in_maps, list(range(N_CORES)), trace=trace)
    dev = np.concatenate([np.asarray(res.results[c]["out"])
                          for c in range(N_CORES)], axis=0)
    full = _assemble(x_rows, dev)
    return full, res


def kernel(x) -> np.ndarray:
    x_np = np.ascontiguousarray(np.asarray(x), dtype=np.float32)
    shape = x_np.shape
    x_np = x_np.reshape(ROWS_TOTAL, D)
    out, _ = _run(x_np, trace=False)
    return out.reshape(*shape[:-1], OUT_D)


# revision 9
# speedup vs baseline: 1.5311x; 1.5311x over previous
"""Taylor feature map kernel for Trainium2 (Bass/Tile), 8-core SPMD.

Input  x:   (2, 16, 2048, 64) f32  -> 65536 rows of dim 64
Output out: (2, 16, 2048, 2145) f32 per row:
    [1, x/D^0.25, x_i^2/(sqrt(D)*sqrt(2)), x_i*x_j/sqrt(D) for i<j (row-major)]

Strategy (v3, ~114us -> target <100us):
- Device emits the 2016 pair products as bf16 (rel-err gate 2e-2 admits it);
  ones/linear/diag (129 of 2145 cols) stay on the host in exact f32; input
  is prescaled bf16.  The kernel is co-limited by the HBM store stream
  (~432GB/s = the 16 DMA engines' aggregate) and the DVE (2x_1p
  tensor_tensor: 2 elem/cycle/lane, ~0.52ns/elem + ~65ns/op).
- CYCLIC-shift decomposition (v3): for s=1..31, out_s[i] = x[i] *
  x[(i+s) mod 64] gives 64 valid products per shift (the wrapped tail
  covers difference 64-s), plus a 32-wide half block for s=32:
  31*64 + 32 = 2016 columns, ZERO pads (v2 had 92), and only 9 DVE ops
  per supertile (v2 had 17):
  * y tile [x|x-wrap (96) | (x>>1)-wrap (96)]: even shifts s read y_even
    at offset s (even -> 4B-aligned); odd shifts s=2t+1 read y_odd at
    offset 2t (even).  One 4D-AP op covers 4 same-parity shifts (out
    row-stride 128 cols, operand row-stride 2), keeping 2x_1p packing
    (2B dtype, stride-1 inner, 4B-aligned sub-rows).
  * DVE payload drops 4.4% (no pad lanes), store bytes drop 4.4%
    (33.03MB vs 34.54MB per core), op overhead nearly halves.
- Pipeline: one cast chain per supertile on the Scalar engine (4 AP
  pieces); input DMAs on the idle GpSimd queue; stores on the Sync queue;
  supertile schedule [2,4,4,6,8x6] primes the store stream by ~15us
  (DVE production rate ~458GB/s now exceeds the stream, so the queue
  stays fed); apool bufs=4 decouples DVE from store-queue lag.
"""

import math
from contextlib import ExitStack

import numpy as np

try:
    import concourse.bass as bass
except ImportError:  # container path for the concourse framework
    import sys

    sys.path.insert(0, "/opt/trn_rl_repo")
    import concourse.bass as bass

import concourse.mybir as mybir
from concourse import tile
from concourse.bass_utils import run_bass_kernel_spmd
from concourse.vector_clock import ScopedClock

MAX_WAITS = 1


class SplitWaitTileContext(tile.TileContext):
    """The stock walrus in this environment rejects instructions carrying
    more than one sync wait ("Too many sync wait commands", observed for
    both TPB_CTRL Drain and DMA_DIRECT2D). Hoist excess waits onto NoOp
    carrier instructions committed just before, on the same engine queue."""

    def _split_waits(self, inst):
        si = getattr(inst, "sync_info", None)
        eng = getattr(inst, "engine", None)
        if (
            si is None
            or not si.on_wait
            or len(si.on_wait) <= MAX_WAITS
            or eng is None
            or eng == mybir.EngineType.Unassigned
        ):
            return None
        waits = list(si.on_wait)
        extra, keep = waits[:-MAX_WAITS], waits[-MAX_WAITS:]
        inst.sync_info = mybir.SyncInfo(on_wait=keep,
                                        on_update=list(si.on_update))
        nops = []
        for i in range(0, len(extra), MAX_WAITS):
            nops.append(mybir.InstNoOp(
                name=self.nc.get_next_instruction_name(),
                sync_info=mybir.SyncInfo(on_wait=extra[i:i + MAX_WAITS],
                                         on_update=[]),
                bass_nofuse=True,
                engine=eng,
            ))
        return nops

    def _commit_instruction(self, inst, lazy_reg_writes=True):
        if isinstance(inst, mybir.Instruction):
            nops = self._split_waits(inst)
            if nops:
                for nop in nops:
                    super()._commit_instruction(nop)
        return super()._commit_instruction(inst, lazy_reg_writes)

    def _drain_and_barrier(self, tick_clock, wait_clock):
        nc = self.nc
        drain_inst = nc.sync.drain()
        wait_clock.add_sem_waits(
            drain_inst.ins, ScopedClock({None: tick_clock.global_clock})
        )
        nops = self._split_waits(drain_inst.ins)
        if nops:
            # _commit path is closed here; append carriers directly, then
            # re-emit a drain that executes after them on the same queue.
            for nop in nops:
                self._add_instruction(nop)
            nc.sync.drain()

        nc.all_engine_barrier()
        assert self.sems is not None
        popped = nc._tile_sem_poison_stack.pop()
        assert popped is self._sem_poison
        nc.clear_and_free_semaphores(list(self.sems.allocated().values()))
        nc.all_engine_barrier()

D = 64
N_CROSS = (D * (D - 1)) // 2  # 2016
OUT_D = 1 + D + D + N_CROSS   # 2145
P = 128
N_CORES = 8
ROWS_TOTAL = 2 * 16 * 2048    # 65536
ROWS_PER_CORE = ROWS_TOTAL // N_CORES  # 8192

RD = math.sqrt(D)                      # 8.0
RRD_INV = 1.0 / math.sqrt(RD)          # 1/D^0.25
DIAG_C = 1.0 / (RD * math.sqrt(2.0))
PRESCALE = 1.0 / math.sqrt(RD)         # y = x*PRESCALE -> y_i*y_j = x_i*x_j/rd

G_ALL = 64                    # row-groups per partition (8192 rows / 128)
G_SCHED = [2, 4, 4, 6, 8, 8, 8, 8, 8, 8]  # supertile heights, sum = 64
assert sum(G_SCHED) == G_ALL

# device column layout: shift-s block at (s-1)*64, width 64 (s=1..31);
# s=32 half block at 1984, width 32.  col (s-1)*64 + i holds
# x_i * x_{(i+s) mod 64}.  Zero pads.
DEV_COLS = 31 * D + 32        # 2016
YW = 192                      # y tile: [x|x-wrap (96) | (x>>1)-wrap (96)]
YO = 96                       # offset of the odd copy

# DVE op groups: 4 same-parity shifts per op (3 for the last even group).
# (i1_off, out_base, c): i1 row-stride 2, out row-stride 128.
OP_GROUPS = []
for _t0 in (0, 4, 8, 12):          # odd shifts s = 2t+1, t = t0..t0+3
    OP_GROUPS.append((YO + 2 * _t0, 128 * _t0, 4))
for _t0, _c in ((1, 4), (5, 4), (9, 4), (13, 3)):  # even shifts s = 2t
    OP_GROUPS.append((2 * _t0, (2 * _t0 - 1) * D, _c))

# host gather map: reference cross column q (triu order) -> device column
_iu, _ju = np.triu_indices(D, k=1)
SRC_COLS = np.empty(N_CROSS, np.int64)
for _q in range(N_CROSS):
    _i, _j = int(_iu[_q]), int(_ju[_q])
    _d = _j - _i
    if _d < 32:
        SRC_COLS[_q] = (_d - 1) * D + _i
    elif _d == 32:
        SRC_COLS[_q] = 1984 + _i
    else:
        SRC_COLS[_q] = (64 - _d - 1) * D + _j


def build_nc() -> bass.Bass:
    nc = bass.Bass()
    x = nc.declare_dram_parameter("x", [P, G_ALL * D], mybir.dt.bfloat16,
                                  isOutput=False)
    out = nc.declare_dram_parameter("out", [ROWS_PER_CORE, DEV_COLS],
                                    mybir.dt.bfloat16, isOutput=True)

    bf16 = mybir.dt.bfloat16
    AF = mybir.ActivationFunctionType

    with SplitWaitTileContext(nc) as tc, ExitStack() as ctx:
        xp = ctx.enter_context(tc.tile_pool(name="xp", bufs=1))
        yp = ctx.enter_context(tc.tile_pool(name="yp", bufs=4))
        apool = ctx.enter_context(tc.tile_pool(name="apool", bufs=3))

        # per-ST input slabs [P, G*64]: one contiguous DMA per partition.
        # GpSimd queue keeps the Scalar queue free for the casts.
        x_tiles = []
        g0 = 0
        for i, g in enumerate(G_SCHED):
            xt = xp.tile([P, g * D], bf16, tag="x", bufs=2, name=f"x_sb{i}")
            nc.gpsimd.dma_start(xt[:], x[:, g0 * D:(g0 + g) * D])
            x_tiles.append(xt)
            g0 += g
        out_v = out.rearrange("(p g) d -> p g d", g=G_ALL)

        g0 = 0
        for st, groups in enumerate(G_SCHED):
            xs = x_tiles[st]
            xs_t = xs[:, 0:1]
            xs_ps = xs_t.ap[0][0]
            # y = [x|x-wrap | (x>>1)-wrap] built by 4 AP pieces on ACT:
            #   y[g, 0:64]    = x[g, 0:64]
            #   y[g, 64:96]   = x[g, 0:32]     (wrap)
            #   y[g, 96:159]  = x[g, 1:64]     (odd copy)
            #   y[g, 159:191] = x[g, 0:32]     (odd wrap)
            y = yp.tile([P, groups, YW], bf16, tag="y")
            y_t = y[:, :, 0:1]
            y_ps = y_t.ap[0][0]
            for y_off, x_off, w in ((0, 0, D), (D, 0, 32),
                                    (YO, 1, D - 1), (YO + D - 1, 0, 32)):
                o = bass.AP(y_t.tensor, y_off,
                            [[y_ps, P], [YW, groups], [1, w]])
                i_ = bass.AP(xs_t.tensor, x_off,
                             [[xs_ps, P], [D, groups], [1, w]])
                nc.scalar.activation(o, i_, AF.Copy)

            a_sb = apool.tile([P, groups, DEV_COLS], bf16, tag="a")
            a_t = a_sb[:, :, 0:1]
            a_ps = a_t.ap[0][0]

            # 8 group ops x 4 (or 3) same-parity shifts
            for i1_off, out_base, c in OP_GROUPS:
                o = bass.AP(a_t.tensor, out_base,
                            [[a_ps, P], [DEV_COLS, groups], [2 * D, c],
                             [1, D]])
                i0 = bass.AP(y_t.tensor, 0,
                             [[y_ps, P], [YW, groups], [0, c], [1, D]])
                i1 = bass.AP(y_t.tensor, i1_off,
                             [[y_ps, P], [YW, groups], [2, c], [1, D]])
                nc.vector.tensor_mul(o, i0, i1)
            # s=32 half block
            o = bass.AP(a_t.tensor, 1984,
                        [[a_ps, P], [DEV_COLS, groups], [1, 32]])
            i0 = bass.AP(y_t.tensor, 0, [[y_ps, P], [YW, groups], [1, 32]])
            i1 = bass.AP(y_t.tensor, 32, [[y_ps, P], [YW, groups], [1, 32]])
            nc.vector.tensor_mul(o, i0, i1)

            nc.sync.dma_start(out_v[:, g0:g0 + groups, :], a_sb[:])
            g0 += groups
    return nc


_NC_CACHE: dict = {}


def _install_ntff_hook_shim():
    """The image's antenv lacks axon_hooks; provide it so trace=True can
    drive NRT profiling via ctypes into libaxon_pjrt.so."""
    import sys as _sys
    import types
    import ctypes
    import contextlib

    if "antenv.axon_hooks" in _sys.modules:
        return
    so_path = "/opt/axon/libaxon_pjrt.so"
    lib = ctypes.CDLL(so_path)
    if not hasattr(lib, "axon_start_nrt_profile"):
        return
    lib.axon_start_nrt_profile.argtypes = [
        ctypes.POINTER(ctypes.c_int64), ctypes.c_size_t]
    lib.axon_start_nrt_profile.restype = ctypes.c_int64
    lib.axon_stop_nrt_profile.argtypes = [ctypes.c_char_p]
    lib.axon_stop_nrt_profile.restype = ctypes.c_int64

    @contextlib.contextmanager
    def _hook(output_dir, device_ids):
        import jax
        jax.devices()
        if device_ids:
            ids = (ctypes.c_int64 * len(device_ids))(*device_ids)
            rc = lib.axon_start_nrt_profile(ids, len(device_ids))
        else:
            rc = lib.axon_start_nrt_profile(None, 0)
        if rc != 0:
            raise RuntimeError(f"axon_start_nrt_profile rc={rc}")
        try:
            yield
        finally:
            n = lib.axon_stop_nrt_profile(str(output_dir).encode())
            print(f"ntff profile: {n} file(s) written to {output_dir}")

    mod = types.ModuleType("antenv.axon_hooks")
    mod.set_axon_ntff_profile_hook = lambda h: None
    mod.get_axon_ntff_profile_hook = lambda: _hook
    _sys.modules["antenv.axon_hooks"] = mod
    import antenv
    antenv.axon_hooks = mod


def _get_nc():
    if "nc" not in _NC_CACHE:
        _NC_CACHE["nc"] = build_nc()
    return _NC_CACHE["nc"]


def _install_loud_cc_hook():
    """Surface the real python traceback when the PJRT compile callback
    fails (the C++ caller swallows it)."""
    from concourse import bass2jax
    bass2jax.install_neuronx_cc_hook()
    try:
        import libneuronxla
    except ImportError:
        return
    if getattr(libneuronxla, "_loud_wrapped", False):
        return
    orig = libneuronxla.neuronx_cc

    def loud_hook(*a, **kw):
        try:
            return orig(*a, **kw)
        except BaseException:
            import traceback
            import sys as _s
            traceback.print_exc()
            _s.stderr.flush()
            raise

    libneuronxla.neuronx_cc = loud_hook
    libneuronxla._loud_wrapped = True
    bass2jax.install_neuronx_cc_hook = lambda: None


def _assemble(x_rows: np.ndarray, dev_rows: np.ndarray) -> np.ndarray:
    """Host assembly: exact f32 ones/linear/diag + permuted bf16 cross."""
    rows = x_rows.shape[0]
    full = np.empty((rows, OUT_D), np.float32)
    full[:, 0] = 1.0
    np.multiply(x_rows, np.float32(RRD_INV), out=full[:, 1:1 + D])
    np.multiply(np.square(x_rows), np.float32(DIAG_C),
                out=full[:, 1 + D:1 + 2 * D])
    # gather in bf16 (cheap), cast on assignment
    full[:, 1 + 2 * D:] = dev_rows[:, SRC_COLS]
    return full


def _run(x_rows: np.ndarray, trace: bool = False):
    """x_rows: [65536, 64] f32 (unscaled). Returns (full_out_rows, res)."""
    _install_loud_cc_hook()
    if trace:
        _install_ntff_hook_shim()
    nc = _get_nc()
    import ml_dtypes
    xc = np.ascontiguousarray(
        (x_rows * np.float32(PRESCALE)).astype(ml_dtypes.bfloat16))
    xs = xc.reshape(N_CORES, P, G_ALL * D)
    in_maps = [{"x": np.ascontiguousarray(xs[c])} for c in range(N_CORES)]
    res = run_bass_kernel_spmd(nc, in_maps, list(range(N_CORES)), trace=trace)
    dev = np.concatenate([np.asarray(res.results[c]["out"])
                          for c in range(N_CORES)], axis=0)
    full = _assemble(x_rows, dev)
    return full, res


def kernel(x) -> np.ndarray:
    x_np = np.ascontiguousarray(np.asarray(x), dtype=np.float32)
    shape = x_np.shape
    x_np = x_np.reshape(ROWS_TOTAL, D)
    out, _ = _run(x_np, trace=False)
    return out.reshape(*shape[:-1], OUT_D)
